# revision 1
# baseline (speedup 1.0000x reference)
"""CapsNet class-capsule dynamic routing kernel for 8x Trainium2 NeuronCores.

Problem: B=256, N_IN=1152, D_IN=8, N_CLS=10, D_OUT=16, 3 routing iters.
Sharding: data-parallel over batch (32 samples/core), W replicated.

Per-core design (local batch BL=32), i = sg*16 + r*4 + i4:
  u_hat matmuls: per (sg, r) one K=32 matmul with a host-built
  block-diagonal stationary lhsT[(i4,d), (i4,b)] = x[b,i,d]*delta(i4)
  at rows 32r (tile_position=(32r,0), 4-way row-tile concurrency, FWL),
  rhs = W stacked [32, (o,c)160]; out = [p=(32*i4+b), 160] in PSUM bank r.
  u_hat SBUF layout: [p = 32*i4 + b, (sg, r, o, c)] fp16 (c innermost
  so every DVE tensor_tensor op runs in 2x_1P mode).
  Routing on DVE (fp16 2x) + PE select-matmuls (lhsT sel[p,b']=(p%32==b')
  contracts partitions: sums i4-quarters while keeping b):
    iter0: s0 = 0.1 * sum_i u_hat via accumulating 3-sg-wide select-MMs.
    iter k: agreement = sum_o u*v (DVE mult + in-place o-tree), bb += agr,
            c = softmax_c(bb) (logits tiny -> no max-sub), s = sum_i c*u
            (DVE mult + accumulating select-MMs, 3 partial col-blocks
            summed before squash), squash -> v.
  Measured ~292-344 us/exec on HW (8 cores); TimelineSim model 292 us.
"""

import numpy as np

B, N_IN, D_IN, N_CLS, D_OUT = 256, 1152, 8, 10, 16
NCORES = 8
BL = B // NCORES          # 32
SG = N_IN // 16           # 72 supergroups
CH = 6                    # supergroups per routing chunk
NCH = SG // CH            # 9 chunks
CO = D_OUT * N_CLS        # 160

_CACHE = {}
USE_GPSIMD = False


def _build_program(loop_n=None, stop_after=None):
    from contextlib import ExitStack

    import concourse.tile as tile
    from concourse import bacc, mybir

    f16 = mybir.dt.float16
    f32 = mybir.dt.float32
    AX = mybir.AxisListType
    OP = mybir.AluOpType
    ACTF = mybir.ActivationFunctionType

    nc = bacc.Bacc("TRN2", target_bir_lowering=False, debug=False, num_devices=1)

    xs_d = nc.dram_tensor("xs", [4, 32, SG, 128], f16, kind="ExternalInput")
    ws_d = nc.dram_tensor("ws", [4, 32, SG, CO], f16, kind="ExternalInput")
    sel_d = nc.dram_tensor("sel", [128, BL], f16, kind="ExternalInput")
    v_d = nc.dram_tensor("v", [BL, D_OUT, N_CLS], f32, kind="ExternalOutput")

    with tile.TileContext(nc) as tc, ExitStack() as ctx:
        persist = ctx.enter_context(tc.tile_pool(name="persist", bufs=1))
        wpool = ctx.enter_context(tc.tile_pool(name="wpool", bufs=2))
        scratch = ctx.enter_context(tc.tile_pool(name="scratch", bufs=3))
        small = ctx.enter_context(tc.tile_pool(name="small", bufs=2))

        # ---- persistent tiles ----
        x_sb = persist.tile([128, SG, 128], f16)
        sel_sb = persist.tile([128, BL], f16)
        sel3_sb = sel_sb
        u_hat = persist.tile([128, SG, 4, D_OUT, N_CLS], f16)
        bb = persist.tile([128, SG, 4, N_CLS], f16)

        # ---- load x and sel ----
        for r in range(4):
            nc.sync.dma_start(x_sb[32 * r:32 * r + 32], xs_d.ap()[r])
        nc.sync.dma_start(sel_sb[:], sel_d.ap())

        loop_cm = tc.For_i(0, loop_n, 1) if loop_n else None
        if loop_cm is not None:
            loop_cm.__enter__()

        # ---- u_hat phase: stream W, 16 packed matmuls per supergroup ----
        uctx = ExitStack()
        upsum_pool = uctx.enter_context(tc.tile_pool(name="ups", bufs=2, space="PSUM"))
        for wc in range(NCH):
            wbuf = wpool.tile([128, CH, CO], f16)
            wsrc = ws_d.ap()[:, :, wc * CH:(wc + 1) * CH]
            for r in range(4):
                nc.sync.dma_start(wbuf[32 * r:32 * r + 32], wsrc[r])
            for s8 in range(CH):
                sg = wc * CH + s8
                ups = upsum_pool.tile([128, 4, 512], f32)
                for r in range(4):
                    nc.tensor.matmul(
                        ups[:, r, 0:CO],
                        x_sb[32 * r:32 * r + 32, sg, :],
                        wbuf[32 * r:32 * r + 32, s8, :],
                        start=True, stop=True,
                        tile_position=(32 * r, 0),
                    )
                usrc = ups[:, :, 0:CO].rearrange("p r (o c) -> p r o c", o=D_OUT)
                if sg % 2 == 0:
                    nc.vector.tensor_copy(u_hat[:, sg], usrc)
                else:
                    nc.scalar.copy(u_hat[:, sg], usrc)

        def sum3(s_ps):
            """Sum the three 160-col accumulation blocks of the batched
            select-matmuls into one [32, CO] f32 tile."""
            s_sum = small.tile([32, CO], f32)
            nc.scalar.copy(s_sum[:], s_ps[0:32, 0:CO])
            nc.vector.tensor_add(s_sum[:], s_sum[:], s_ps[0:32, CO:2 * CO])
            nc.vector.tensor_add(s_sum[:], s_sum[:], s_ps[0:32, 2 * CO:3 * CO])
            return s_sum

        def squash(s_in, scale, want_f32):
            """s_in: [32, CO] f32 AP. Returns (v_sb f16, v32 or None)."""
            s_sc = small.tile([32, CO], f32)
            nc.scalar.mul(s_sc[:], s_in, scale)
            s2 = small.tile([32, CO], f32)
            nc.vector.tensor_mul(s2[:], s_sc[:], s_sc[:])
            sq = small.tile([32, N_CLS], f32)
            nc.vector.tensor_reduce(
                sq[:], s2[:].rearrange("p (o c) -> p c o", o=D_OUT),
                axis=AX.X, op=OP.add)
            t = small.tile([32, N_CLS], f32)
            nc.scalar.activation(t[:], sq[:], ACTF.Sqrt)
            tp = small.tile([32, N_CLS], f32)
            nc.vector.tensor_scalar_add(tp[:], t[:], 1e-8)
            q1 = small.tile([32, N_CLS], f32)
            nc.vector.tensor_scalar_add(q1[:], sq[:], 1.0)
            den = small.tile([32, N_CLS], f32)
            nc.vector.tensor_mul(den[:], q1[:], tp[:])
            rden = small.tile([32, N_CLS], f32)
            nc.vector.reciprocal(rden[:], den[:])
            sc = small.tile([32, N_CLS], f32)
            nc.vector.tensor_mul(sc[:], sq[:], rden[:])
            sc_b = sc[:].unsqueeze(1).to_broadcast([32, D_OUT, N_CLS])
            s_v = s_sc[:].rearrange("p (o c) -> p o c", o=D_OUT)
            v_sb = small.tile([32, D_OUT, N_CLS], f16)
            nc.vector.tensor_mul(v_sb[:], s_v, sc_b)
            v32 = None
            if want_f32:
                v32 = small.tile([32, D_OUT, N_CLS], f32)
                nc.vector.tensor_mul(v32[:], s_v, sc_b)
            return v_sb, v32

        uctx.close()

        if stop_after == "uhat":
            vdump = small.tile([32, D_OUT, N_CLS], f32)
            nc.vector.tensor_copy(vdump[:], u_hat[0:32, 0, 0])
            nc.sync.dma_start(v_d.ap(), vdump[:])

        # ---- iter 0: s0 = 0.1 * sum_i u_hat ----
        iters = () if stop_after == "uhat" else ((1,) if stop_after == "iter1" else (1, 2))
        spsum_pool = ctx.enter_context(tc.tile_pool(name="sps", bufs=1, space="PSUM"))
        if stop_after == "uhat":
            iters = ()
        s_ps = spsum_pool.tile([32, 512], f32, name="s_ps0", tag="s_ps") if stop_after != "uhat" else None
        n_mm = (SG // 3) * 4
        k = 0
        for sg in (range(0, SG, 3) if stop_after != "uhat" else ()):
            for r in range(4):
                rhs = u_hat[:, sg:sg + 3, r]
                nc.tensor.matmul(
                    s_ps[0:32, 0:3 * CO], sel3_sb[:], rhs,
                    start=(k == 0), stop=(k == n_mm - 1))
                k += 1
        if stop_after != "uhat":
            v_sb, v32 = squash(sum3(s_ps)[:], 1.0 / N_CLS, stop_after == "iter0")
            if stop_after == "iter0":
                nc.sync.dma_start(v_d.ap(), v32[:])
            # v_exp: replicate v to all 4 partition groups
            v_exp = small.tile([128, D_OUT, N_CLS], f16)
            for q in range(4):
                nc.sync.dma_start(v_exp[32 * q:32 * q + 32], v_sb[:])
        if stop_after in ("uhat", "iter0"):
            iters = ()
        for it in iters:
            # ---- agreement pass (chunked): agr = sum_o u*v ; bb += agr ----
            for chk in range(NCH):
                sl = slice(chk * CH, (chk + 1) * CH)
                eng = nc.gpsimd if (USE_GPSIMD and chk >= NCH - 2) else nc.vector
                u_ch = u_hat[:, sl]
                v_bb = (v_exp[:].unsqueeze(1).unsqueeze(1)
                        .to_broadcast([128, CH, 4, D_OUT, N_CLS]))
                prod = scratch.tile([128, CH, 4, D_OUT, N_CLS], f16)
                eng.tensor_mul(prod[:], u_ch, v_bb)
                eng.tensor_add(prod[:, :, :, 0:8], prod[:, :, :, 0:8],
                               prod[:, :, :, 8:16])
                eng.tensor_add(prod[:, :, :, 0:4], prod[:, :, :, 0:4],
                               prod[:, :, :, 4:8])
                eng.tensor_add(prod[:, :, :, 0:2], prod[:, :, :, 0:2],
                               prod[:, :, :, 2:4])
                if it == 1:
                    eng.tensor_add(bb[:, sl], prod[:, :, :, 0], prod[:, :, :, 1])
                else:
                    eng.tensor_add(prod[:, :, :, 0], prod[:, :, :, 0],
                                   prod[:, :, :, 1])
                    eng.tensor_add(bb[:, sl], bb[:, sl], prod[:, :, :, 0])
            # ---- softmax over c (logits are small: skip max-subtraction) ----
            e = small.tile([128, SG, 4, N_CLS], f16)
            nc.scalar.activation(e[:], bb[:], ACTF.Exp)
            z = small.tile([128, SG * 4], f32)
            nc.vector.tensor_reduce(
                z[:], e[:].rearrange("p s r c -> p (s r) c"),
                axis=AX.X, op=OP.add)
            rz32 = small.tile([128, SG * 4], f32)
            nc.vector.reciprocal(rz32[:], z[:])
            rz = small.tile([128, SG * 4], f16)
            nc.vector.tensor_copy(rz[:], rz32[:])
            rz_b = (rz[:].rearrange("p (s r) -> p s r", s=SG).unsqueeze(-1)
                    .to_broadcast([128, SG, 4, N_CLS]))
            nc.vector.tensor_mul(e[:], e[:], rz_b)
            cw = e
            # ---- s pass (chunked mult + accumulating select-matmuls) ----
            s_ps = spsum_pool.tile([32, 512], f32)
            k = 0
            for chk in range(NCH):
                sl = slice(chk * CH, (chk + 1) * CH)
                eng = nc.gpsimd if (USE_GPSIMD and chk >= NCH - 2) else nc.vector
                cw_b = (cw[:, sl].unsqueeze(3)
                        .to_broadcast([128, CH, 4, D_OUT, N_CLS]))
                prod2 = scratch.tile([128, CH, 4, D_OUT, N_CLS], f16)
                eng.tensor_mul(prod2[:], u_hat[:, sl], cw_b)
                for s8 in range(0, CH, 3):
                    w3 = min(3, CH - s8)
                    for r in range(4):
                        nc.tensor.matmul(
                            s_ps[0:32, 0:w3 * CO], sel3_sb[:, 0:32],
                            prod2[:, s8:s8 + w3, r],
                            start=(k == 0), stop=(k == n_mm - 1))
                        k += 1
            # ---- squash ----
            last = it == 2 or stop_after == "iter1"
            v_sb, v32 = squash(sum3(s_ps)[:], 1.0, last)
            if last:
                nc.sync.dma_start(v_d.ap(), v32[:])
            elif it == 1:
                v_exp = small.tile([128, D_OUT, N_CLS], f16)
                for q in range(4):
                    nc.sync.dma_start(v_exp[32 * q:32 * q + 32], v_sb[:])

        if loop_cm is not None:
            loop_cm.__exit__(None, None, None)

    nc.compile()
    return nc


def _get_program(loop_n=None, stop_after=None):
    key = ("nc", loop_n, stop_after)
    if key not in _CACHE:
        _CACHE[key] = _build_program(loop_n, stop_after)
    return _CACHE[key]


def _prep_inputs(x, W):
    """Host-side layout prep. Returns per-core input maps."""
    sel = (np.arange(128)[:, None] % 32 == np.arange(BL)[None, :]).astype(np.float16)
    # i = sg*16 + r*4 + i4 ; lhsT for (sg, r) is a [32=(i4,d), 128=(i4,b)]
    # block-diagonal of xT; rhs is W stacked [32=(i4,d), 160=(o,c)].
    Wr = np.asarray(W[0]).reshape(SG, 4, 4, N_CLS, D_OUT, D_IN)  # sg r i4 c o d
    ws = np.ascontiguousarray(
        Wr.transpose(1, 2, 5, 0, 4, 3)                     # r i4 d sg o c
    ).astype(np.float16).reshape(4, 32, SG, CO)
    in_maps = []
    for c in range(NCORES):
        xl = np.asarray(x[c * BL:(c + 1) * BL])            # [32, 1152, 8]
        xr = xl.reshape(BL, SG, 4, 4, D_IN)                # b sg r i4 d
        m = xr.transpose(2, 3, 4, 1, 0).astype(np.float16)  # r i4 d sg b
        xbd = np.zeros((4, 4, D_IN, SG, 4, BL), np.float16)
        for q in range(4):
            xbd[:, q, :, :, q, :] = m[:, q]
        xsc = np.ascontiguousarray(xbd).reshape(4, 32, SG, 128)
        in_maps.append({"xs": xsc, "ws": ws, "sel": sel})
    return in_maps


def kernel(x, W):
    from concourse.bass_utils import run_bass_kernel_spmd

    nc = _get_program()
    in_maps = _prep_inputs(x, W)
    res = run_bass_kernel_spmd(nc, in_maps, core_ids=list(range(NCORES)))
    outs = []
    for c in range(NCORES):
        v = res.results[c]["v"]                  # [32, 16, 10]
        outs.append(v.transpose(0, 2, 1))        # [32, 10, 16]
    return np.ascontiguousarray(np.concatenate(outs, axis=0)).astype(np.float32)

